# revision 27
# baseline (speedup 1.0000x reference)
"""MultiHeadAttention with RoPE on 8 Trainium2 NeuronCores.

Sharding: batch (2) x head-group (4 heads each) -> 8 cores. Each core
computes q/k/v projections for its 4 heads of one batch element, causal
attention, and a partial output projection (row-shard of Wo). The host
sums the 4 partial outputs per batch element (the "all-reduce").

Pipelined design: the sequence is processed in 4 token slices of 512.
Per slice tt: DMA x(tt) (prefetched), Q/K/V projections + RoPE, then
causal attention for q-tile tt against k-blocks 0..4tt+3, then (after
the next slice's projections are issued, to hide the normalize chain)
the output projection for slice tt. This keeps the PE array busy
end-to-end instead of phase-serializing.

Precision: x and all weights travel as bf16 (halves DMA); Q/K + scores
stay fp32(r); exp outputs and V tiles are bf16; PSUM accumulation fp32
throughout.
"""

import numpy as np
import ml_dtypes

import concourse.bacc as bacc
import concourse.mybir as mybir
import concourse.tile as tile
from concourse.bass_utils import run_bass_kernel_spmd

F32 = mybir.dt.float32
F32R = mybir.dt.float32r
BF16 = mybir.dt.bfloat16
EXP = mybir.ActivationFunctionType.Exp

B, S, D = 2, 2048, 1024
H, HD = 16, 64
THETA = 10000.0
NCORES = 8
NH = 4          # heads per core
C = NH * HD     # 256 channels per core
P = 128
DC = D // P     # 8 contraction chunks
NSL = 4         # token slices (= q tiles)
SL = S // NSL   # 512
NTB = S // P    # 16 token blocks

_NC_CACHE = None
LAST_RESULTS = None


def _build():
    nc = bacc.Bacc(None)

    xT = nc.dram_tensor("xT", [D, S], BF16, kind="ExternalInput")
    wqT = nc.dram_tensor("wqT", [D, C], BF16, kind="ExternalInput")
    wkT = nc.dram_tensor("wkT", [D, C], BF16, kind="ExternalInput")
    wvT = nc.dram_tensor("wvT", [D, C], BF16, kind="ExternalInput")
    woT = nc.dram_tensor("woT", [C, D], BF16, kind="ExternalInput")
    cosP = nc.dram_tensor("cosP", [P, S], F32, kind="ExternalInput")
    sinP = nc.dram_tensor("sinP", [P, S], F32, kind="ExternalInput")
    out = nc.dram_tensor("out", [S, D], F32, kind="ExternalOutput")
    dscr = nc.dram_tensor("dscr", [16, SL], F32, kind="Internal")
    dscr2 = dscr.rearrange("n (tb p) -> n p tb", p=P)

    xT3 = xT.rearrange("(dc di) t -> di dc t", di=P)
    wq3 = wqT.rearrange("(dc di) c -> di dc c", di=P)
    wk3 = wkT.rearrange("(dc di) c -> di dc c", di=P)
    wv3 = wvT.rearrange("(dc di) c -> di dc c", di=P)
    wo3 = woT.rearrange("(cp ci) o -> ci cp o", ci=P)

    XOR1 = [i ^ 1 for i in range(32)]
    HS = (slice(0, 64), slice(64, 128))

    with tile.TileContext(nc) as tc:
        with (
            tc.tile_pool(name="cn", bufs=1) as cn,
            tc.tile_pool(name="big", bufs=1) as big,
            tc.tile_pool(name="etp", bufs=3) as etp,
            tc.tile_pool(name="shp", bufs=4) as shp,
            tc.tile_pool(name="nrm", bufs=2) as nrm,
        ):
            # ---- persistent tiles ----
            wq_sb = cn.tile([P, DC, C], BF16, tag="wq")
            wk_sb = cn.tile([P, DC, C], BF16, tag="wk")
            wv_sb = cn.tile([P, DC, C], BF16, tag="wv")
            wo_sb = cn.tile([P, 2, D], BF16, tag="wo")
            cos_sb = cn.tile([P, S], F32, tag="cos")
            sin_sb = cn.tile([P, S], F32, tag="sin")
            xt_sb = [cn.tile([P, DC, SL], BF16, tag=f"xt{t}", name=f"xt{t}")
                     for t in range(NSL)]
            qk = {(pj, pr): big.tile([P, S], F32R, tag=f"{pj}{pr}", name=f"{pj}{pr}")
                  for pj in "qk" for pr in range(2)}
            vp = [big.tile([P, NH, 65], BF16, tag=f"vp{i}", name=f"vp{i}")
                  for i in range(NTB)]
            yt = [big.tile([P, S], BF16, tag=f"yt{pr}", name=f"yt{pr}")
                  for pr in range(2)]

            # ---- all input DMAs up front, priority-ordered ----
            def dma_x(tt):
                ts = slice(tt * SL, (tt + 1) * SL)
                for dc in range(DC):
                    nc.sync.dma_start(xt_sb[tt][:, dc, :], xT3[:, dc, ts])

            def dma_cs(tt):
                for hh in range(2):
                    cs_ = slice(tt * SL + hh * 256, tt * SL + (hh + 1) * 256)
                    nc.sync.dma_start(cos_sb[:, cs_], cosP[:, cs_])
                    nc.sync.dma_start(sin_sb[:, cs_], sinP[:, cs_])

            for dc in range(DC):
                nc.sync.dma_start(wk_sb[:, dc, :], wk3[:, dc, :])
            for dc in range(DC):
                nc.sync.dma_start(wq_sb[:, dc, :], wq3[:, dc, :])
            dma_x(0)
            dma_cs(0)
            for dc in range(DC):
                nc.sync.dma_start(wv_sb[:, dc, :], wv3[:, dc, :])
            dma_x(1)
            for cp in range(2):
                for hh in range(2):
                    os_ = slice(hh * 512, (hh + 1) * 512)
                    nc.sync.dma_start(wo_sb[:, cp, os_], wo3[:, cp, os_])
            dma_cs(1)
            dma_x(2)
            dma_cs(2)
            dma_x(3)
            dma_cs(3)

            for i in range(NTB):
                nc.gpsimd.memset(vp[i][:, :, 64:65], 1.0)

            # ---- pipeline stages ----
            def proj(tt):
                """QKV projections + rope for token slice tt."""
                ts = slice(tt * SL, (tt + 1) * SL)
                with (
                    tc.tile_pool(name=f"psP{tt}", bufs=2, space="PSUM") as psP,
                    tc.tile_pool(name=f"psV{tt}", bufs=2, space="PSUM") as psV,
                ):
                    for pj, wsb in (("k", wk_sb), ("q", wq_sb)):
                        for pr in range(2):
                            ps = psP.tile([P, SL], F32, tag="p")
                            for dc in range(DC):
                                nc.tensor.matmul(
                                    ps[:], wsb[:, dc, pr * P:(pr + 1) * P],
                                    xt_sb[tt][:, dc, :],
                                    start=(dc == 0), stop=(dc == DC - 1))
                            sh = shp.tile([P, SL], F32, tag="sh")
                            t1 = shp.tile([P, SL], F32, tag="t1")
                            nc.vector.stream_shuffle(sh[:], ps[:], XOR1)
                            nc.vector.tensor_mul(t1[:], ps[:], cos_sb[:, ts])
                            nc.gpsimd.tensor_mul(sh[:], sh[:], sin_sb[:, ts])
                            nc.gpsimd.tensor_add(qk[(pj, pr)][:, ts], t1[:], sh[:])
                    for tb in range(4):
                        gtb = 4 * tt + tb
                        bs = slice(tb * P, (tb + 1) * P)
                        psv = psV.tile([P, C], F32, tag="v")
                        for dc in range(DC):
                            nc.tensor.matmul(
                                psv[:], xt_sb[tt][:, dc, bs], wv_sb[:, dc, :],
                                start=(dc == 0), stop=(dc == DC - 1))
                        nc.vector.tensor_copy(
                            vp[gtb][:, :, 0:HD],
                            psv.rearrange("p (h c) -> p h c", c=HD))

            def attn(qt):
                """Causal attention for q tile qt (both head pairs)."""
                nkb = 4 * qt + 4
                qs = slice(qt * SL, (qt + 1) * SL)
                with (
                    tc.tile_pool(name=f"psS{qt}", bufs=2, space="PSUM") as psS,
                    tc.tile_pool(name=f"psA{qt}", bufs=4, space="PSUM") as psA,
                ):
                    for pr in range(2):
                        qtile = qk[("q", pr)]
                        ktile = qk[("k", pr)]
                        av = [psA.tile([P, SL], F32, tag="av", name=f"av{qt}{pr}{o}")
                              for o in range(2)]

                        def flush_av(entry, av=av, pr=pr, nkb=nkb):
                            kb, off, w, et = entry
                            for o in range(2):
                                nc.tensor.matmul(
                                    av[o][0:65, off:SL],
                                    vp[kb][:, 2 * pr + o, :],
                                    et[:, o * SL:o * SL + w],
                                    start=(kb == 0), stop=(kb == nkb - 1),
                                    skip_group_check=True)

                        pend = []
                        for kb in range(nkb):
                            j = kb - 4 * qt
                            off = max(0, j * P)
                            w = SL - off
                            sc = psS.tile([P, 2 * SL], F32, tag="sc")
                            for o in range(2):
                                nc.tensor.matmul(
                                    sc[:, o * SL:o * SL + w],
                                    ktile[HS[o], kb * P:(kb + 1) * P],
                                    qtile[HS[o], qt * SL + off:(qt + 1) * SL],
                                    start=True, stop=True)
                            et = etp.tile([P, 2 * SL], BF16, tag="et")
                            if w == SL:
                                nc.scalar.activation(
                                    et[:, 0:2 * SL], sc[:, 0:2 * SL],
                                    EXP, scale=0.125)
                            else:
                                for o in range(2):
                                    nc.scalar.activation(
                                        et[:, o * SL:o * SL + w],
                                        sc[:, o * SL:o * SL + w],
                                        EXP, scale=0.125)
                            if j >= 0:
                                for o in range(2):
                                    nc.gpsimd.affine_select(
                                        et[:, o * SL:o * SL + P],
                                        et[:, o * SL:o * SL + P],
                                        [[1, P]], mybir.AluOpType.is_ge, 0.0,
                                        base=0, channel_multiplier=-1)
                            pend.append((kb, off, w, et))
                            if len(pend) > 1:
                                flush_av(pend.pop(0))
                        while pend:
                            flush_av(pend.pop(0))

                        for o in range(2):
                            # 1/denom via a tiny DRAM-transpose round trip:
                            # DVE reciprocal is ~6 cyc/elem, so run it on a
                            # [128, 4] layout (free=4) instead of [1, 512].
                            # Keeps DVE nearly free here so the next slice's
                            # rope (queued behind on DVE) starts sooner.
                            si = 4 * qt + 2 * pr + o
                            den = nrm.tile([1, SL], F32, tag="den")
                            nc.vector.tensor_copy(den[:], av[o][64:65, :])
                            nc.sync.dma_start(dscr[si:si + 1, :], den[:])
                            denT = nrm.tile([P, 4], F32, tag="denT")
                            nc.sync.dma_start(denT[:], dscr2[si, :, :])
                            recT = nrm.tile([P, 4], F32, tag="recT")
                            nc.vector.reciprocal(recT[:], denT[:])
                            nc.sync.dma_start(dscr2[si, :, :], recT[:])
                            rec = nrm.tile([1, SL], F32, tag="rec")
                            nc.sync.dma_start(rec[:], dscr[si:si + 1, :])
                            rb = nrm.tile([64, SL], F32, tag="rb")
                            nc.gpsimd.partition_broadcast(rb[:], rec[:])
                            nc.vector.tensor_mul(
                                yt[pr][64 * o:64 * o + 64, qs],
                                av[o][0:64, :], rb[:])

            def outproj(tt):
                """Output projection for slice tt."""
                with tc.tile_pool(name=f"psO{tt}", bufs=4, space="PSUM") as psO:
                    for tb in range(4):
                        gtb = 4 * tt + tb
                        tbs = slice(gtb * P, (gtb + 1) * P)
                        for oc in range(2):
                            po = psO.tile([P, 512], F32, tag="po")
                            for cp in range(2):
                                nc.tensor.matmul(
                                    po[:], yt[cp][:, tbs],
                                    wo_sb[:, cp, oc * 512:(oc + 1) * 512],
                                    start=(cp == 0), stop=(cp == 1))
                            ot = shp.tile([P, 512], F32, tag="ot", name="ot")
                            if oc == 0:
                                nc.vector.tensor_copy(ot[:], po[:])
                            else:
                                nc.scalar.copy(ot[:], po[:])
                            for hh in range(2):
                                ocs = slice(oc * 512 + hh * 256,
                                            oc * 512 + (hh + 1) * 256)
                                nc.sync.dma_start(
                                    out[tbs, ocs], ot[:, hh * 256:(hh + 1) * 256])

            proj(0)
            for tt in range(NSL):
                attn(tt)
                if tt + 1 < NSL:
                    proj(tt + 1)
                outproj(tt)

    nc.finalize()
    return nc


def _prep_core_inputs(x, pos, Wq, Wk, Wv, Wo):
    """Per-core input dicts (host-side sharding + layout prep)."""
    inv_freq = THETA ** (-np.arange(0, HD, 2, dtype=np.float32) / HD)
    ang = pos.astype(np.float32)[:, None] * inv_freq[None, :]   # (S, 32)
    cos = np.cos(ang).astype(np.float32)
    sin = np.sin(ang).astype(np.float32)
    p = np.arange(P)
    pairidx = (p % HD) // 2
    cosP = np.ascontiguousarray(cos[:, pairidx].T)              # (128, S)
    sgn = np.where(p % 2 == 0, -1.0, 1.0).astype(np.float32)
    sinP = np.ascontiguousarray(sin[:, pairidx].T * sgn[:, None])

    bf = ml_dtypes.bfloat16
    xTs = [np.ascontiguousarray(x[b].T).astype(bf) for b in range(B)]
    maps = []
    for c in range(NCORES):
        b, g = divmod(c, NH)
        cs = slice(C * g, C * (g + 1))
        maps.append({
            "xT": xTs[b],
            "wqT": np.ascontiguousarray(Wq[cs, :].T).astype(bf),
            "wkT": np.ascontiguousarray(Wk[cs, :].T).astype(bf),
            "wvT": np.ascontiguousarray(Wv[cs, :].T).astype(bf),
            "woT": np.ascontiguousarray(Wo[:, cs].T).astype(bf),
            "cosP": cosP,
            "sinP": sinP,
        })
    return maps


def kernel(in_features, token_positions, Wq, Wk, Wv, Wo):
    global _NC_CACHE, LAST_RESULTS
    x = np.asarray(in_features, dtype=np.float32)
    pos = np.asarray(token_positions)
    Wq = np.asarray(Wq, dtype=np.float32)
    Wk = np.asarray(Wk, dtype=np.float32)
    Wv = np.asarray(Wv, dtype=np.float32)
    Wo = np.asarray(Wo, dtype=np.float32)

    if _NC_CACHE is None:
        _NC_CACHE = _build()
    maps = _prep_core_inputs(x, pos, Wq, Wk, Wv, Wo)
    res = run_bass_kernel_spmd(_NC_CACHE, maps, core_ids=list(range(NCORES)))
    LAST_RESULTS = res
    parts = [r["out"] for r in res.results]
    outb = [parts[4 * b] + parts[4 * b + 1] + parts[4 * b + 2] + parts[4 * b + 3]
            for b in range(B)]
    return np.stack(outb).astype(np.float32)


if __name__ == "__main__":
    rng = np.random.default_rng(0)
    x = rng.standard_normal((B, S, D), dtype=np.float32)
    o = kernel(x, np.arange(S, dtype=np.int32),
               *(rng.standard_normal((D, D), dtype=np.float32) / 32
                 for _ in range(4)))
    print(o.shape, o.dtype)


# revision 30
# speedup vs baseline: 1.2606x; 1.2606x over previous
"""MultiHeadAttention with RoPE on 8 Trainium2 NeuronCores.

Sharding: batch (2) x head-group (4 heads each) -> 8 cores. Each core
computes q/k/v projections for its 4 heads of one batch element, causal
attention, and a partial output projection (row-shard of Wo). The host
sums the 4 partial outputs per batch element (the "all-reduce").

Pipelined design: the sequence is processed in 4 token slices of 512.
Per slice tt: DMA x(tt) (prefetched), Q/K/V projections + RoPE, then
causal attention for q-tile tt against k-blocks 0..4tt+3, then (after
the next slice's projections are issued, to hide the normalize chain)
the output projection for slice tt. This keeps the PE array busy
end-to-end instead of phase-serializing.

Precision: x and all weights travel as bf16 (halves DMA); Q/K + scores
stay fp32(r); exp outputs and V tiles are bf16; PSUM accumulation fp32
throughout.
"""

import numpy as np
import ml_dtypes

import concourse.bacc as bacc
import concourse.mybir as mybir
import concourse.tile as tile
from concourse.bass_utils import run_bass_kernel_spmd

F32 = mybir.dt.float32
F32R = mybir.dt.float32r
BF16 = mybir.dt.bfloat16
EXP = mybir.ActivationFunctionType.Exp

B, S, D = 2, 2048, 1024
H, HD = 16, 64
THETA = 10000.0
NCORES = 8
NH = 4          # heads per core
C = NH * HD     # 256 channels per core
P = 128
DC = D // P     # 8 contraction chunks
NSL = 4         # token slices (= q tiles)
SL = S // NSL   # 512
NTB = S // P    # 16 token blocks

_NC_CACHE = None
LAST_RESULTS = None


def _build():
    nc = bacc.Bacc(None)

    xT = nc.dram_tensor("xT", [D, S], BF16, kind="ExternalInput")
    wqT = nc.dram_tensor("wqT", [D, C], BF16, kind="ExternalInput")
    wkT = nc.dram_tensor("wkT", [D, C], BF16, kind="ExternalInput")
    wvT = nc.dram_tensor("wvT", [D, C], BF16, kind="ExternalInput")
    woT = nc.dram_tensor("woT", [C, D], BF16, kind="ExternalInput")
    cosP = nc.dram_tensor("cosP", [P, S], F32, kind="ExternalInput")
    sinP = nc.dram_tensor("sinP", [P, S], F32, kind="ExternalInput")
    out = nc.dram_tensor("out", [S, D], F32, kind="ExternalOutput")

    xT3 = xT.rearrange("(dc di) t -> di dc t", di=P)
    wq3 = wqT.rearrange("(dc di) c -> di dc c", di=P)
    wk3 = wkT.rearrange("(dc di) c -> di dc c", di=P)
    wv3 = wvT.rearrange("(dc di) c -> di dc c", di=P)
    wo3 = woT.rearrange("(cp ci) o -> ci cp o", ci=P)

    XOR1 = [i ^ 1 for i in range(32)]
    HS = (slice(0, 64), slice(64, 128))

    with tile.TileContext(nc) as tc:
        with (
            tc.tile_pool(name="cn", bufs=1) as cn,
            tc.tile_pool(name="big", bufs=1) as big,
            tc.tile_pool(name="etp", bufs=4) as etp,
            tc.tile_pool(name="shp", bufs=4) as shp,
            tc.tile_pool(name="nrm", bufs=2) as nrm,
        ):
            # ---- persistent tiles ----
            wq_sb = cn.tile([P, DC, C], BF16, tag="wq")
            wk_sb = cn.tile([P, DC, C], BF16, tag="wk")
            wv_sb = cn.tile([P, DC, C], BF16, tag="wv")
            wo_sb = cn.tile([P, 2, D], BF16, tag="wo")
            cos_sb = cn.tile([P, S], F32, tag="cos")
            sin_sb = cn.tile([P, S], F32, tag="sin")
            xt_sb = [cn.tile([P, DC, SL], BF16, tag=f"xt{t}", name=f"xt{t}")
                     for t in range(NSL)]
            qk = {(pj, pr): big.tile([P, S], F32R, tag=f"{pj}{pr}", name=f"{pj}{pr}")
                  for pj in "qk" for pr in range(2)}
            vp = [big.tile([P, NH, 65], BF16, tag=f"vp{i}", name=f"vp{i}")
                  for i in range(NTB)]
            yt = [big.tile([P, S], BF16, tag=f"yt{pr}", name=f"yt{pr}")
                  for pr in range(2)]

            # ---- all input DMAs up front, priority-ordered ----
            def dma_x(tt):
                ts = slice(tt * SL, (tt + 1) * SL)
                for dc in range(DC):
                    nc.sync.dma_start(xt_sb[tt][:, dc, :], xT3[:, dc, ts])

            def dma_cs(tt):
                for hh in range(2):
                    cs_ = slice(tt * SL + hh * 256, tt * SL + (hh + 1) * 256)
                    nc.sync.dma_start(cos_sb[:, cs_], cosP[:, cs_])
                    nc.sync.dma_start(sin_sb[:, cs_], sinP[:, cs_])

            for dc in range(DC):
                nc.sync.dma_start(wq_sb[:, dc, :], wq3[:, dc, :])
            for dc in range(DC):
                nc.sync.dma_start(wk_sb[:, dc, :], wk3[:, dc, :])
            dma_x(0)
            dma_cs(0)
            for dc in range(DC):
                nc.sync.dma_start(wv_sb[:, dc, :], wv3[:, dc, :])
            dma_x(1)
            for cp in range(2):
                for hh in range(2):
                    os_ = slice(hh * 512, (hh + 1) * 512)
                    nc.sync.dma_start(wo_sb[:, cp, os_], wo3[:, cp, os_])
            dma_cs(1)
            dma_x(2)
            dma_cs(2)
            dma_x(3)
            dma_cs(3)

            for i in range(NTB):
                nc.gpsimd.memset(vp[i][:, :, 64:65], 1.0)

            # ---- pipeline stages ----
            def proj(tt):
                """QKV projections + rope for token slice tt."""
                ts = slice(tt * SL, (tt + 1) * SL)
                with (
                    tc.tile_pool(name=f"psP{tt}", bufs=2, space="PSUM") as psP,
                    tc.tile_pool(name=f"psV{tt}", bufs=2, space="PSUM") as psV,
                ):
                    for pj, wsb in (("k", wk_sb), ("q", wq_sb)):
                        for pr in range(2):
                            ps = psP.tile([P, SL], F32, tag="p")
                            for dc in range(DC):
                                nc.tensor.matmul(
                                    ps[:], wsb[:, dc, pr * P:(pr + 1) * P],
                                    xt_sb[tt][:, dc, :],
                                    start=(dc == 0), stop=(dc == DC - 1))
                            sh = shp.tile([P, SL], F32, tag="sh")
                            t1 = shp.tile([P, SL], F32, tag="t1")
                            nc.vector.stream_shuffle(sh[:], ps[:], XOR1)
                            nc.vector.tensor_mul(t1[:], ps[:], cos_sb[:, ts])
                            nc.gpsimd.tensor_mul(sh[:], sh[:], sin_sb[:, ts])
                            nc.gpsimd.tensor_add(qk[(pj, pr)][:, ts], t1[:], sh[:])
                    for tb in range(4):
                        gtb = 4 * tt + tb
                        bs = slice(tb * P, (tb + 1) * P)
                        psv = psV.tile([P, C], F32, tag="v")
                        for dc in range(DC):
                            nc.tensor.matmul(
                                psv[:], xt_sb[tt][:, dc, bs], wv_sb[:, dc, :],
                                start=(dc == 0), stop=(dc == DC - 1))
                        nc.vector.tensor_copy(
                            vp[gtb][:, :, 0:HD],
                            psv.rearrange("p (h c) -> p h c", c=HD))

            def attn(qt):
                """Causal attention for q tile qt (both head pairs)."""
                nkb = 4 * qt + 4
                qs = slice(qt * SL, (qt + 1) * SL)
                with (
                    tc.tile_pool(name=f"psS{qt}", bufs=2, space="PSUM") as psS,
                    tc.tile_pool(name=f"psA{qt}", bufs=4, space="PSUM") as psA,
                ):
                    for pr in range(2):
                        qtile = qk[("q", pr)]
                        ktile = qk[("k", pr)]
                        av = [psA.tile([P, SL], F32, tag="av", name=f"av{qt}{pr}{o}")
                              for o in range(2)]

                        def flush_av(entry, av=av, pr=pr, nkb=nkb):
                            kb, off, w, et = entry
                            for o in range(2):
                                nc.tensor.matmul(
                                    av[o][0:65, off:SL],
                                    vp[kb][:, 2 * pr + o, :],
                                    et[:, o * SL:o * SL + w],
                                    start=(kb == 0), stop=(kb == nkb - 1),
                                    skip_group_check=True)

                        pend = []
                        for kb in range(nkb):
                            j = kb - 4 * qt
                            off = max(0, j * P)
                            w = SL - off
                            sc = psS.tile([P, 2 * SL], F32, tag="sc")
                            for o in range(2):
                                nc.tensor.matmul(
                                    sc[:, o * SL:o * SL + w],
                                    ktile[HS[o], kb * P:(kb + 1) * P],
                                    qtile[HS[o], qt * SL + off:(qt + 1) * SL],
                                    start=True, stop=True)
                            et = etp.tile([P, 2 * SL], BF16, tag="et")
                            if w == SL:
                                nc.scalar.activation(
                                    et[:, 0:2 * SL], sc[:, 0:2 * SL],
                                    EXP, scale=0.125)
                            else:
                                for o in range(2):
                                    nc.scalar.activation(
                                        et[:, o * SL:o * SL + w],
                                        sc[:, o * SL:o * SL + w],
                                        EXP, scale=0.125)
                            if j >= 0:
                                for o in range(2):
                                    nc.gpsimd.affine_select(
                                        et[:, o * SL:o * SL + P],
                                        et[:, o * SL:o * SL + P],
                                        [[1, P]], mybir.AluOpType.is_ge, 0.0,
                                        base=0, channel_multiplier=-1)
                            pend.append((kb, off, w, et))
                            if len(pend) > 2:
                                flush_av(pend.pop(0))
                        while pend:
                            flush_av(pend.pop(0))

                        for o in range(2):
                            rec = nrm.tile([1, SL], F32, tag="rec")
                            nc.vector.reciprocal(rec[:], av[o][64:65, :])
                            rb = nrm.tile([64, SL], F32, tag="rb")
                            nc.gpsimd.partition_broadcast(rb[:], rec[:])
                            nc.vector.tensor_mul(
                                yt[pr][64 * o:64 * o + 64, qs],
                                av[o][0:64, :], rb[:])

            def outproj(tt):
                """Output projection for slice tt."""
                with tc.tile_pool(name=f"psO{tt}", bufs=4, space="PSUM") as psO:
                    for tb in range(4):
                        gtb = 4 * tt + tb
                        tbs = slice(gtb * P, (gtb + 1) * P)
                        for oc in range(2):
                            po = psO.tile([P, 512], F32, tag="po")
                            for cp in range(2):
                                nc.tensor.matmul(
                                    po[:], yt[cp][:, tbs],
                                    wo_sb[:, cp, oc * 512:(oc + 1) * 512],
                                    start=(cp == 0), stop=(cp == 1))
                            ot = shp.tile([P, 512], F32, tag="ot", name="ot")
                            if oc == 0:
                                nc.vector.tensor_copy(ot[:], po[:])
                            else:
                                nc.scalar.copy(ot[:], po[:])
                            for hh in range(2):
                                ocs = slice(oc * 512 + hh * 256,
                                            oc * 512 + (hh + 1) * 256)
                                nc.sync.dma_start(
                                    out[tbs, ocs], ot[:, hh * 256:(hh + 1) * 256])

            proj(0)
            for tt in range(NSL):
                attn(tt)
                if tt + 1 < NSL:
                    proj(tt + 1)
                outproj(tt)

    nc.finalize()
    return nc


def _prep_core_inputs(x, pos, Wq, Wk, Wv, Wo):
    """Per-core input dicts (host-side sharding + layout prep)."""
    inv_freq = THETA ** (-np.arange(0, HD, 2, dtype=np.float32) / HD)
    ang = pos.astype(np.float32)[:, None] * inv_freq[None, :]   # (S, 32)
    cos = np.cos(ang).astype(np.float32)
    sin = np.sin(ang).astype(np.float32)
    p = np.arange(P)
    pairidx = (p % HD) // 2
    cosP = np.ascontiguousarray(cos[:, pairidx].T)              # (128, S)
    sgn = np.where(p % 2 == 0, -1.0, 1.0).astype(np.float32)
    sinP = np.ascontiguousarray(sin[:, pairidx].T * sgn[:, None])

    bf = ml_dtypes.bfloat16
    xTs = [np.ascontiguousarray(x[b].T).astype(bf) for b in range(B)]
    maps = []
    for c in range(NCORES):
        b, g = divmod(c, NH)
        cs = slice(C * g, C * (g + 1))
        maps.append({
            "xT": xTs[b],
            "wqT": np.ascontiguousarray(Wq[cs, :].T).astype(bf),
            "wkT": np.ascontiguousarray(Wk[cs, :].T).astype(bf),
            "wvT": np.ascontiguousarray(Wv[cs, :].T).astype(bf),
            "woT": np.ascontiguousarray(Wo[:, cs].T).astype(bf),
            "cosP": cosP,
            "sinP": sinP,
        })
    return maps


def kernel(in_features, token_positions, Wq, Wk, Wv, Wo):
    global _NC_CACHE, LAST_RESULTS
    x = np.asarray(in_features, dtype=np.float32)
    pos = np.asarray(token_positions)
    Wq = np.asarray(Wq, dtype=np.float32)
    Wk = np.asarray(Wk, dtype=np.float32)
    Wv = np.asarray(Wv, dtype=np.float32)
    Wo = np.asarray(Wo, dtype=np.float32)

    if _NC_CACHE is None:
        _NC_CACHE = _build()
    maps = _prep_core_inputs(x, pos, Wq, Wk, Wv, Wo)
    res = run_bass_kernel_spmd(_NC_CACHE, maps, core_ids=list(range(NCORES)))
    LAST_RESULTS = res
    parts = [r["out"] for r in res.results]
    outb = [parts[4 * b] + parts[4 * b + 1] + parts[4 * b + 2] + parts[4 * b + 3]
            for b in range(B)]
    return np.stack(outb).astype(np.float32)


if __name__ == "__main__":
    rng = np.random.default_rng(0)
    x = rng.standard_normal((B, S, D), dtype=np.float32)
    o = kernel(x, np.arange(S, dtype=np.int32),
               *(rng.standard_normal((D, D), dtype=np.float32) / 32
                 for _ in range(4)))
    print(o.shape, o.dtype)


# revision 33
# speedup vs baseline: 1.3280x; 1.0535x over previous
"""MultiHeadAttention with RoPE on 8 Trainium2 NeuronCores.

Sharding: batch (2) x head-group (4 heads each) -> 8 cores. Each core
computes q/k/v projections for its 4 heads of one batch element, causal
attention, and a partial output projection (row-shard of Wo). The host
sums the 4 partial outputs per batch element (the "all-reduce").

Pipelined design: the sequence is processed in 4 token slices of 512.
Per slice tt: DMA x(tt) (prefetched), Q/K/V projections + RoPE, then
causal attention for q-tile tt against k-blocks 0..4tt+3, then (after
the next slice's projections are issued, to hide the normalize chain)
the output projection for slice tt. This keeps the PE array busy
end-to-end instead of phase-serializing.

Precision: x and all weights travel as bf16 (halves DMA); Q/K + scores
stay fp32(r); exp outputs and V tiles are bf16; PSUM accumulation fp32
throughout.
"""

import numpy as np
import ml_dtypes

import concourse.bacc as bacc
import concourse.mybir as mybir
import concourse.tile as tile
from concourse.bass_utils import run_bass_kernel_spmd

F32 = mybir.dt.float32
F32R = mybir.dt.float32r
BF16 = mybir.dt.bfloat16
EXP = mybir.ActivationFunctionType.Exp

B, S, D = 2, 2048, 1024
H, HD = 16, 64
THETA = 10000.0
NCORES = 8
NH = 4          # heads per core
C = NH * HD     # 256 channels per core
P = 128
DC = D // P     # 8 contraction chunks
NSL = 4         # token slices (= q tiles)
SL = S // NSL   # 512
NTB = S // P    # 16 token blocks

_NC_CACHE = None
LAST_RESULTS = None


def _build():
    nc = bacc.Bacc(None)

    xT = nc.dram_tensor("xT", [D, S], BF16, kind="ExternalInput")
    wqT = nc.dram_tensor("wqT", [D, C], BF16, kind="ExternalInput")
    wkT = nc.dram_tensor("wkT", [D, C], BF16, kind="ExternalInput")
    wvT = nc.dram_tensor("wvT", [D, C], BF16, kind="ExternalInput")
    woT = nc.dram_tensor("woT", [C, D], BF16, kind="ExternalInput")
    cosP = nc.dram_tensor("cosP", [P, S], F32, kind="ExternalInput")
    sinP = nc.dram_tensor("sinP", [P, S], F32, kind="ExternalInput")
    out = nc.dram_tensor("out", [S, D], F32, kind="ExternalOutput")

    xT3 = xT.rearrange("(dc di) t -> di dc t", di=P)
    wq3 = wqT.rearrange("(dc di) c -> di dc c", di=P)
    wk3 = wkT.rearrange("(dc di) c -> di dc c", di=P)
    wv3 = wvT.rearrange("(dc di) c -> di dc c", di=P)
    wo3 = woT.rearrange("(cp ci) o -> ci cp o", ci=P)

    XOR1 = [i ^ 1 for i in range(32)]
    HS = (slice(0, 64), slice(64, 128))

    with tile.TileContext(nc) as tc:
        with (
            tc.tile_pool(name="cn", bufs=1) as cn,
            tc.tile_pool(name="big", bufs=1) as big,
            tc.tile_pool(name="etp", bufs=3) as etp,
            tc.tile_pool(name="shp", bufs=4) as shp,
            tc.tile_pool(name="nrm", bufs=2) as nrm,
        ):
            # ---- persistent tiles ----
            wq_sb = cn.tile([P, DC, C], BF16, tag="wq")
            wk_sb = cn.tile([P, DC, C], BF16, tag="wk")
            wv_sb = cn.tile([P, DC, C], BF16, tag="wv")
            wo_sb = cn.tile([P, 2, D], BF16, tag="wo")
            cos_sb = cn.tile([P, S], F32, tag="cos")
            sin_sb = cn.tile([P, S], F32, tag="sin")
            xt_sb = [cn.tile([P, DC, SL], BF16, tag=f"xt{t}", name=f"xt{t}")
                     for t in range(NSL)]
            qk = {(pj, pr): big.tile([P, S], F32R, tag=f"{pj}{pr}", name=f"{pj}{pr}")
                  for pj in "qk" for pr in range(2)}
            vp = [big.tile([P, NH, 65], BF16, tag=f"vp{i}", name=f"vp{i}")
                  for i in range(NTB)]
            yt = [big.tile([P, S], BF16, tag=f"yt{pr}", name=f"yt{pr}")
                  for pr in range(2)]

            # ---- all input DMAs up front, priority-ordered ----
            def dma_x(tt):
                ts = slice(tt * SL, (tt + 1) * SL)
                for dc in range(DC):
                    nc.sync.dma_start(xt_sb[tt][:, dc, :], xT3[:, dc, ts])

            def dma_cs(tt):
                for hh in range(2):
                    cs_ = slice(tt * SL + hh * 256, tt * SL + (hh + 1) * 256)
                    nc.sync.dma_start(cos_sb[:, cs_], cosP[:, cs_])
                    nc.sync.dma_start(sin_sb[:, cs_], sinP[:, cs_])

            for dc in range(DC):
                nc.sync.dma_start(wq_sb[:, dc, :], wq3[:, dc, :])
            for dc in range(DC):
                nc.sync.dma_start(wk_sb[:, dc, :], wk3[:, dc, :])
            dma_x(0)
            dma_cs(0)
            for dc in range(DC):
                nc.sync.dma_start(wv_sb[:, dc, :], wv3[:, dc, :])
            dma_x(1)
            for cp in range(2):
                for hh in range(2):
                    os_ = slice(hh * 512, (hh + 1) * 512)
                    nc.sync.dma_start(wo_sb[:, cp, os_], wo3[:, cp, os_])
            dma_cs(1)
            dma_x(2)
            dma_cs(2)
            dma_x(3)
            dma_cs(3)

            for i in range(NTB):
                nc.gpsimd.memset(vp[i][:, :, 64:65], 1.0)

            # ---- pipeline stages ----
            def proj(tt):
                """QKV projections + rope for token slice tt."""
                ts = slice(tt * SL, (tt + 1) * SL)
                with (
                    tc.tile_pool(name=f"psP{tt}", bufs=2, space="PSUM") as psP,
                    tc.tile_pool(name=f"psV{tt}", bufs=2, space="PSUM") as psV,
                ):
                    for pj, wsb in (("k", wk_sb), ("q", wq_sb)):
                        for pr in range(2):
                            ps = psP.tile([P, SL], F32, tag="p")
                            for dc in range(DC):
                                nc.tensor.matmul(
                                    ps[:], wsb[:, dc, pr * P:(pr + 1) * P],
                                    xt_sb[tt][:, dc, :],
                                    start=(dc == 0), stop=(dc == DC - 1))
                            sh = shp.tile([P, SL], F32, tag="sh")
                            t1 = shp.tile([P, SL], F32, tag="t1")
                            nc.vector.stream_shuffle(sh[:], ps[:], XOR1)
                            nc.vector.tensor_mul(t1[:], ps[:], cos_sb[:, ts])
                            nc.gpsimd.tensor_mul(sh[:], sh[:], sin_sb[:, ts])
                            nc.gpsimd.tensor_add(qk[(pj, pr)][:, ts], t1[:], sh[:])
                    for tb in range(4):
                        gtb = 4 * tt + tb
                        bs = slice(tb * P, (tb + 1) * P)
                        psv = psV.tile([P, C], F32, tag="v")
                        for dc in range(DC):
                            nc.tensor.matmul(
                                psv[:], xt_sb[tt][:, dc, bs], wv_sb[:, dc, :],
                                start=(dc == 0), stop=(dc == DC - 1))
                        nc.vector.tensor_copy(
                            vp[gtb][:, :, 0:HD],
                            psv.rearrange("p (h c) -> p h c", c=HD))

            def attn(qt):
                """Causal attention for q tile qt (both head pairs)."""
                nkb = 4 * qt + 4
                qs = slice(qt * SL, (qt + 1) * SL)
                with (
                    tc.tile_pool(name=f"psS{qt}", bufs=2, space="PSUM") as psS,
                    tc.tile_pool(name=f"psA{qt}", bufs=4, space="PSUM") as psA,
                ):
                    avs = {}
                    for pr in range(2):
                        qtile = qk[("q", pr)]
                        ktile = qk[("k", pr)]
                        av = [psA.tile([P, SL], F32, tag="av", name=f"av{qt}{pr}{o}")
                              for o in range(2)]
                        avs[pr] = av

                        def flush_av(entry, av=av, pr=pr, nkb=nkb):
                            kb, off, w, et = entry
                            for o in range(2):
                                nc.tensor.matmul(
                                    av[o][0:65, off:SL],
                                    vp[kb][:, 2 * pr + o, :],
                                    et[:, o * SL:o * SL + w],
                                    start=(kb == 0), stop=(kb == nkb - 1),
                                    skip_group_check=True)

                        pend = []
                        for kb in range(nkb):
                            j = kb - 4 * qt
                            off = max(0, j * P)
                            w = SL - off
                            sc = psS.tile([P, 2 * SL], F32, tag="sc")
                            for o in range(2):
                                nc.tensor.matmul(
                                    sc[:, o * SL:o * SL + w],
                                    ktile[HS[o], kb * P:(kb + 1) * P],
                                    qtile[HS[o], qt * SL + off:(qt + 1) * SL],
                                    start=True, stop=True)
                            et = etp.tile([P, 2 * SL], BF16, tag="et")
                            if w == SL:
                                nc.scalar.activation(
                                    et[:, 0:2 * SL], sc[:, 0:2 * SL],
                                    EXP, scale=0.125)
                            else:
                                for o in range(2):
                                    nc.scalar.activation(
                                        et[:, o * SL:o * SL + w],
                                        sc[:, o * SL:o * SL + w],
                                        EXP, scale=0.125)
                            if j >= 0:
                                for o in range(2):
                                    nc.gpsimd.affine_select(
                                        et[:, o * SL:o * SL + P],
                                        et[:, o * SL:o * SL + P],
                                        [[1, P]], mybir.AluOpType.is_ge, 0.0,
                                        base=0, channel_multiplier=-1)
                            pend.append((kb, off, w, et))
                            if len(pend) > 1:
                                flush_av(pend.pop(0))
                        while pend:
                            flush_av(pend.pop(0))

                    # Normalize both pairs only after both kb loops: the
                    # epilogue is DVE/Pool-only and the kb loops use no DVE,
                    # so this changes nothing except the Pool queue order --
                    # pair 1's affine_select masks no longer sit behind
                    # pair 0's partition_broadcast (which waits ~3.4us on
                    # the reciprocal chain).
                    for pr in range(2):
                        for o in range(2):
                            rec = nrm.tile([1, SL], F32, tag="rec")
                            nc.vector.reciprocal(rec[:], avs[pr][o][64:65, :])
                            rb = nrm.tile([64, SL], F32, tag="rb")
                            nc.gpsimd.partition_broadcast(rb[:], rec[:])
                            nc.vector.tensor_mul(
                                yt[pr][64 * o:64 * o + 64, qs],
                                avs[pr][o][0:64, :], rb[:])

            def outproj(tt):
                """Output projection for slice tt."""
                with tc.tile_pool(name=f"psO{tt}", bufs=4, space="PSUM") as psO:
                    for tb in range(4):
                        gtb = 4 * tt + tb
                        tbs = slice(gtb * P, (gtb + 1) * P)
                        for oc in range(2):
                            po = psO.tile([P, 512], F32, tag="po")
                            for cp in range(2):
                                nc.tensor.matmul(
                                    po[:], yt[cp][:, tbs],
                                    wo_sb[:, cp, oc * 512:(oc + 1) * 512],
                                    start=(cp == 0), stop=(cp == 1))
                            ot = shp.tile([P, 512], F32, tag="ot", name="ot")
                            if oc == 0:
                                nc.vector.tensor_copy(ot[:], po[:])
                            else:
                                nc.scalar.copy(ot[:], po[:])
                            for hh in range(2):
                                ocs = slice(oc * 512 + hh * 256,
                                            oc * 512 + (hh + 1) * 256)
                                nc.sync.dma_start(
                                    out[tbs, ocs], ot[:, hh * 256:(hh + 1) * 256])

            proj(0)
            for tt in range(NSL):
                attn(tt)
                if tt + 1 < NSL:
                    proj(tt + 1)
                outproj(tt)

    nc.finalize()
    return nc


def _prep_core_inputs(x, pos, Wq, Wk, Wv, Wo):
    """Per-core input dicts (host-side sharding + layout prep)."""
    inv_freq = THETA ** (-np.arange(0, HD, 2, dtype=np.float32) / HD)
    ang = pos.astype(np.float32)[:, None] * inv_freq[None, :]   # (S, 32)
    cos = np.cos(ang).astype(np.float32)
    sin = np.sin(ang).astype(np.float32)
    p = np.arange(P)
    pairidx = (p % HD) // 2
    cosP = np.ascontiguousarray(cos[:, pairidx].T)              # (128, S)
    sgn = np.where(p % 2 == 0, -1.0, 1.0).astype(np.float32)
    sinP = np.ascontiguousarray(sin[:, pairidx].T * sgn[:, None])

    bf = ml_dtypes.bfloat16
    xTs = [np.ascontiguousarray(x[b].T).astype(bf) for b in range(B)]
    maps = []
    for c in range(NCORES):
        b, g = divmod(c, NH)
        cs = slice(C * g, C * (g + 1))
        maps.append({
            "xT": xTs[b],
            "wqT": np.ascontiguousarray(Wq[cs, :].T).astype(bf),
            "wkT": np.ascontiguousarray(Wk[cs, :].T).astype(bf),
            "wvT": np.ascontiguousarray(Wv[cs, :].T).astype(bf),
            "woT": np.ascontiguousarray(Wo[:, cs].T).astype(bf),
            "cosP": cosP,
            "sinP": sinP,
        })
    return maps


def kernel(in_features, token_positions, Wq, Wk, Wv, Wo):
    global _NC_CACHE, LAST_RESULTS
    x = np.asarray(in_features, dtype=np.float32)
    pos = np.asarray(token_positions)
    Wq = np.asarray(Wq, dtype=np.float32)
    Wk = np.asarray(Wk, dtype=np.float32)
    Wv = np.asarray(Wv, dtype=np.float32)
    Wo = np.asarray(Wo, dtype=np.float32)

    if _NC_CACHE is None:
        _NC_CACHE = _build()
    maps = _prep_core_inputs(x, pos, Wq, Wk, Wv, Wo)
    res = run_bass_kernel_spmd(_NC_CACHE, maps, core_ids=list(range(NCORES)))
    LAST_RESULTS = res
    parts = [r["out"] for r in res.results]
    outb = [parts[4 * b] + parts[4 * b + 1] + parts[4 * b + 2] + parts[4 * b + 3]
            for b in range(B)]
    return np.stack(outb).astype(np.float32)


if __name__ == "__main__":
    rng = np.random.default_rng(0)
    x = rng.standard_normal((B, S, D), dtype=np.float32)
    o = kernel(x, np.arange(S, dtype=np.int32),
               *(rng.standard_normal((D, D), dtype=np.float32) / 32
                 for _ in range(4)))
    print(o.shape, o.dtype)
